# revision 7
# baseline (speedup 1.0000x reference)
"""Trainium2 Bass kernel for nn_AttentionSimple (sparse_attention, 8 cores).

Reference (per batch row b):
    e      = embeddings[k[b]]              # [S, E] gather
    scores = q[b] . e[s]                   # [S]
    attn   = softmax(scores); ctx = sum_s attn[s] * e[s]
    out    = ctx @ W.T + b                 # [B, 2]

Algorithm: count-weighted vocab-space softmax - no per-token gathers.
Scores depend on s only through v = k[b, s], so group softmax terms by
vocabulary id:
    c[b, v]  = |{s : k[b, s] = v}|         (histogram of k, built host-side
                                            during input sharding)
    l[b, v]  = q[b] . embeddings[v]        (dense PE matmul, fp16 inputs)
    A        = exp(l)                      (ACT, fp32 -> bf16)
    out[b]   = (sum_v c*A*EW[v]) / (sum_v c*A[b,v]),  EW = emb @ W.T + b

Sharding: padded vocabulary (51200 = 8 x 50 chunks of 128) split across 8
cores; each core handles all 128 batch rows for its 6400 vocab entries.
Each core returns partial numerators/denominators; host sums and divides.

Per-core pipeline (v2):
  - mm1: 25 pair matmuls. lhsT = fp16 pair block (embedding rows 0:50 =
    even chunk, 50:100 = odd chunk), rhs = block-diagonal [qT|qT] fp16,
    N=256 -> one matmul yields two chunks' logits for all 128 batches.
  - ACT: exp over 3-PSUM-bank spans (1536 cols) -> bf16 A tiles; big
    tiles amortize the ~293ns per-instruction overhead.
  - DVE: A *= counts (fp8_e4m3 transport - exact for counts <= 15).
  - mm2: acc[9, 512] += st9_quad.T @ A (bf16), 13 accumulating matmuls.
  - DMA: 7 dispatches total (fatter transfers; the v1 13-dispatch stream
    ran at 148 GB/s, dispatch-serialization-limited).
  - Emission order interleaves mm1 groups ahead of mm2 so the PE queue
    (strict in-order) never stalls on the ACT->DVE chain.
"""

import numpy as np

BATCH, SEQ, EMB, VOCAB, OUT = 128, 8192, 50, 50000, 2
N_CORES = 8
CSH = 50                         # vocab chunks per core
NCHUNK = CSH * N_CORES           # 400
VPAD = NCHUNK * 128              # 51200
VSH = CSH * 128                  # 6400
NPAIR = CSH // 2                 # 25
NQUAD = 13                       # 12 full quads + 1 pair-quad
GROUPS = [6, 6, 6, 6, 1]         # pairs per processing group

_CACHE = {}


def _build_nc():
    from contextlib import ExitStack

    import concourse.mybir as mybir
    import concourse.tile as tile
    from concourse import bacc

    f32 = mybir.dt.float32
    f16 = mybir.dt.float16
    bf16 = mybir.dt.bfloat16
    fp8 = mybir.dt.float8e4
    nc = bacc.Bacc("TRN2", target_bir_lowering=False, debug=False,
                   num_devices=N_CORES)

    et_d = nc.dram_tensor("et", [100, NPAIR * 128], f16, kind="ExternalInput")
    qw_d = nc.dram_tensor("qw", [128, 256], f16, kind="ExternalInput")
    st_d = nc.dram_tensor("st", [128, NQUAD * 9], bf16, kind="ExternalInput")
    ct_d = nc.dram_tensor("ct", [128, CSH * BATCH], fp8, kind="ExternalInput")
    o_d = nc.dram_tensor("o", [9, 4 * BATCH], f32, kind="ExternalOutput")

    with tile.TileContext(nc) as tc, ExitStack() as ctx:
        const_p = ctx.enter_context(tc.tile_pool(name="const", bufs=1))
        ps_p = ctx.enter_context(tc.tile_pool(name="ps", bufs=2, space="PSUM"))
        tail_p = ctx.enter_context(tc.tile_pool(name="tailps", bufs=1,
                                                space="PSUM"))
        acc_p = ctx.enter_context(tc.tile_pool(name="acc", bufs=1,
                                               space="PSUM"))

        # SBUF tiles (one tile per DMA so dependency tracking stays exact)
        wtile = const_p.tile([128, 256], f16)
        qw_sb = const_p.tile([128, 256], f16)
        st_sb = const_p.tile([128, NQUAD * 9], bf16)
        etA = const_p.tile([128, 768], f16)     # pairs 0-5
        etB = const_p.tile([128, 768], f16)     # pairs 6-11
        etC = const_p.tile([128, 1664], f16)    # pairs 12-24
        ctA = const_p.tile([128, 3072], fp8)    # groups 0-1
        ctB = const_p.tile([128, 3328], fp8)    # groups 2-4
        a_tiles = [const_p.tile([128, g * 256], bf16, name=f"a{i}")
                   for i, g in enumerate(GROUPS)]

        # DVE init: zero the warm-up tile and the unused contraction rows
        # (96:128, 32-aligned; DMA then fills 96:100) of the et tiles --
        # garbage there could be NaN and the
        # PE multiplies it by qw's zero rows (NaN * 0 = NaN).
        nc.vector.memset(wtile[:].bitcast(f32), 0.0)
        nc.vector.memset(etA[96:128, :].bitcast(f32), 0.0)
        nc.vector.memset(etB[96:128, :].bitcast(f32), 0.0)
        nc.vector.memset(etC[96:128, :].bitcast(f32), 0.0)

        # Scalar queue: small params
        nc.scalar.dma_start(qw_sb[:], qw_d.ap())
        nc.scalar.dma_start(st_sb[:], st_d.ap())

        # Sync queue: bulk transfers in consumption order
        nc.sync.dma_start(etA[0:100, :], et_d.ap()[:, 0:768])
        nc.sync.dma_start(etB[0:100, :], et_d.ap()[:, 768:1536])
        nc.sync.dma_start(ctA[:], ct_d.ap()[:, 0:3072])
        nc.sync.dma_start(etC[0:100, :], et_d.ap()[:, 1536:3200])
        nc.sync.dma_start(ctB[:], ct_d.ap()[:, 3072:6400])

        # PE warm-up while the first DMAs land
        wps = tail_p.tile([128, 256], f32, tag="tail")
        for _ in range(3):
            nc.tensor.matmul(wps[:], lhsT=wtile[:, 0:128], rhs=wtile[:],
                             start=True, stop=True)

        acc = acc_p.tile([9, 4 * BATCH], f32)

        et_of_group = [etA, etB, etC, etC, etC]
        et_col0 = [0, 0, 0, 768, 1536]          # col offset within its tile
        ct_of_group = [ctA, ctA, ctB, ctB, ctB]
        ct_col0 = [0, 1536, 0, 1536, 3072]

        ps_tiles = [None] * len(GROUPS)

        def emit_mm1(g):
            npairs = GROUPS[g]
            if g == len(GROUPS) - 1:
                ps = tail_p.tile([128, 256], f32, tag="tail")
            else:
                ps = ps_p.tile([128, 1536], f32, tag="ps")
            ps_tiles[g] = ps
            et = et_of_group[g]
            c0 = et_col0[g]
            for p in range(npairs):
                nc.tensor.matmul(
                    ps[:, p * 256:(p + 1) * 256],
                    lhsT=et[:, c0 + p * 128:c0 + (p + 1) * 128],
                    rhs=qw_sb[:],
                    start=True, stop=True,
                )

        quad_idx = 0

        def emit_tail(g):
            nonlocal quad_idx
            npairs = GROUPS[g]
            ncols = npairs * 256
            ps = ps_tiles[g]
            a = a_tiles[g]
            nc.scalar.activation(a[:], ps[:, 0:ncols],
                                 mybir.ActivationFunctionType.Exp)
            ct = ct_of_group[g]
            c0 = ct_col0[g]
            nc.vector.tensor_mul(a[:], a[:], ct[:, c0:c0 + ncols])
            nquads = (npairs + 1) // 2
            for j in range(nquads):
                n = min(512, ncols - j * 512)
                nc.tensor.matmul(
                    acc[:, 0:n],
                    lhsT=st_sb[:, quad_idx * 9:(quad_idx + 1) * 9],
                    rhs=a[:, j * 512:j * 512 + n],
                    start=(quad_idx == 0), stop=(quad_idx == NQUAD - 1),
                    skip_group_check=True,
                )
                quad_idx += 1

        # Interleave: PE program order keeps mm1 ahead of the mm2 that
        # depends on the ACT->DVE chain.
        emit_mm1(0)
        emit_mm1(1)
        emit_tail(0)
        emit_mm1(2)
        emit_tail(1)
        emit_mm1(3)
        emit_tail(2)
        emit_mm1(4)
        emit_tail(3)
        emit_tail(4)

        osb = const_p.tile([9, 4 * BATCH], f32)
        nc.scalar.activation(osb[:], acc[:],
                             mybir.ActivationFunctionType.Copy)
        nc.scalar.dma_start(o_d.ap(), osb[:])

    nc.finalize()
    return nc


def _prep_inputs(q, k, embeddings, W, b):
    import ml_dtypes

    q = np.ascontiguousarray(q, dtype=np.float32)
    emb = np.ascontiguousarray(embeddings, dtype=np.float32)
    W = np.ascontiguousarray(W, dtype=np.float32)
    b = np.ascontiguousarray(b, dtype=np.float32)
    k = np.asarray(k)

    embT = np.zeros((EMB, VPAD), np.float32)
    embT[:, :VOCAB] = emb.T

    qw = np.zeros((128, 256), np.float16)
    qw[0:EMB, 0:BATCH] = q.T
    qw[EMB:2 * EMB, BATCH:2 * BATCH] = q.T

    EWp = np.zeros((VPAD, OUT), np.float32)
    EWp[:VOCAB] = emb @ W.T + b[None, :]

    flat = (np.arange(BATCH, dtype=np.int64)[:, None] * VPAD
            + k.astype(np.int64)).ravel()
    C = np.bincount(flat, minlength=BATCH * VPAD).reshape(BATCH, VPAD)
    assert C.max() <= 15, "count histogram overflows fp8_e4m3 exact range"
    C = C.astype(np.float32)

    in_maps = []
    for core in range(N_CORES):
        v0 = core * VSH
        blocks = embT[:, v0:v0 + VSH].reshape(EMB, CSH, 128)
        e2 = np.zeros((100, NPAIR * 128), np.float16)
        e2[0:EMB] = blocks[:, 0::2, :].reshape(EMB, NPAIR * 128)
        e2[EMB:2 * EMB] = blocks[:, 1::2, :].reshape(EMB, NPAIR * 128)

        ew = EWp[v0:v0 + VSH].reshape(CSH, 128, OUT)
        st = np.zeros((128, NQUAD, 9), np.float32)
        for qd in range(12):
            for j in range(4):
                st[:, qd, 2 * j:2 * j + 2] = ew[4 * qd + j]
            st[:, qd, 8] = 1.0
        st[:, 12, 0:2] = ew[48]
        st[:, 12, 2:4] = ew[49]
        st[:, 12, 8] = 1.0
        st = np.ascontiguousarray(
            st.reshape(128, NQUAD * 9)).astype(ml_dtypes.bfloat16)

        ct = np.ascontiguousarray(
            C[:, v0:v0 + VSH].reshape(BATCH, CSH, 128)
            .transpose(2, 1, 0).reshape(128, CSH * BATCH)
            .astype(ml_dtypes.float8_e4m3fn))
        in_maps.append({"et": np.ascontiguousarray(e2), "qw": qw,
                        "st": st, "ct": ct})
    return in_maps


def _run_device(in_maps, **kwargs):
    from concourse.bass_utils import run_bass_kernel_spmd

    if "nc" not in _CACHE:
        _CACHE["nc"] = _build_nc()
    return run_bass_kernel_spmd(_CACHE["nc"], in_maps,
                                core_ids=list(range(N_CORES)), **kwargs)


def _unshard(res):
    P = np.zeros((9, 4 * BATCH), np.float64)
    for i in range(N_CORES):
        P += res.results[i]["o"].astype(np.float64)
    numer = np.zeros((OUT, BATCH), np.float64)
    denom = np.zeros(BATCH, np.float64)
    for j in range(4):
        numer += P[2 * j:2 * j + 2, j * BATCH:(j + 1) * BATCH]
        denom += P[8, j * BATCH:(j + 1) * BATCH]
    out = (numer / denom[None, :]).T
    return np.ascontiguousarray(out, dtype=np.float32)


def kernel(q, k, embeddings, W, b, **_unused):
    in_maps = _prep_inputs(q, k, embeddings, W, b)
    res = _run_device(in_maps)
    return _unshard(res)
